# revision 48
# baseline (speedup 1.0000x reference)
"""Trainium2 Bass kernel for DiagonalSSMLayer.

Math: y = C_w @ h + D*u  where  h[l] = lam*h[l-1] + (B_w @ u)[l]  (per state
channel, lam = sigmoid(log_lambda)).  The reference computes the causal
exponential-decay convolution via FFT; here it is the exact linear recurrence,
done with the DVE's native tensor_tensor_scan.

Sharding: 8 cores = (batch b in 0..3) x (sequence half s in 0..1).
Each core gets u[b, s*2048:(s+1)*2048, :] transposed to [D=1024, 2048] so the
contraction dim d sits on SBUF partitions for both GEMMs (out = lhsT.T @ rhs
contracts over the partition dim).  All HBM traffic is bf16: matmuls run at
the same 1 cycle/row as f32r, DMA bytes halve, and the rel-err budget (2e-2)
comfortably absorbs the ~4e-3 bf16 noise.

Cross-half carry: second-half cores prepend a HALO of the last `HALO`
positions of the first half and run the scan through it, which reconstructs
the incoming state up to a factor lam^HALO <= 6e-3 of the carried state
(measured end-to-end contribution < 1e-4).  First-half cores get a zero halo,
keeping the program uniform across cores (SPMD).

Schedule notes (from perfetto traces):
 - The 16 DMA queues drain descriptors roughly in trigger order, so the
   sync (SP) queue issues the input stream in consumption order: BwT/halo
   k-halves interleaved, then chunk1..chunk2, CwT (GEMM2-only), chunk3,
   chunk4.  4-8KB per-partition descriptors are the service sweet spot --
   both finer splits and coarser merges measured slower.  Tiny params ride
   the gpsimd SWDGE path as ONE packed transfer.
 - GPSIMD cannot access PSUM (and rejects scalar_tensor_tensor outright),
   so the scans and the y-materialize scalar_tensor_tensor ops all run on
   Vector (~33us busy, just under the ~35us Tensor roofline).
 - PE warmup matmuls bridge the initial DMA wait and raise the HAM clock
   out of the low p-state before the real GEMM stream starts.
 - y leaves per chunk as one 8KB/partition DMA, except the last chunk
   which streams per-k to shorten the tail.
"""

import numpy as np
import ml_dtypes

BF16 = ml_dtypes.bfloat16

B, L, DM, NS = 4, 4096, 1024, 256
HALF = L // 2          # 2048 sequence positions per core
NCORES = 8
LC = 512               # l-chunk (matmul free dim / scan chunk)
NLC = HALF // LC       # 4 main chunks
HALO = 512
NHC = HALO // LC       # 1 halo chunk
NTOT = NHC + NLC       # u chunks incl. halo
KT = DM // 128         # 8 k-tiles (contraction over d)
NT = NS // 128         # 2 n-tiles (state channels)

_CACHE = {}


def _build(warm=10, hks=2):
    from concourse import bacc, tile, mybir

    MULT = mybir.AluOpType.mult
    ADD = mybir.AluOpType.add
    f32 = mybir.dt.float32
    bf16 = mybir.dt.bfloat16

    nc = bacc.Bacc("TRN2", target_bir_lowering=False, debug=False,
                   num_devices=NCORES)

    # chunk-major u so each chunk's DMA is one contiguous 8KB/partition run
    uT_d = nc.dram_tensor("uT", [128, NTOT, KT, LC], bf16, kind="ExternalInput").ap()
    BwT_d = nc.dram_tensor("BwT", [128, KT, NS], bf16, kind="ExternalInput").ap()
    CwT_d = nc.dram_tensor("CwT", [128, NT, DM], bf16, kind="ExternalInput").ap()
    # packed small params: cols [0..NT) = lam per n-tile, [NT..NT+KT) = D per k
    par_d = nc.dram_tensor("params", [128, NT + KT], f32, kind="ExternalInput").ap()
    yT_d = nc.dram_tensor("yT", [128, NLC, KT, LC], bf16, kind="ExternalOutput").ap()

    with tile.TileContext(nc) as tc:
        with tc.tile_pool(name="const", bufs=1) as cpool, \
             tc.tile_pool(name="u", bufs=1) as upool, \
             tc.tile_pool(name="h", bufs=1) as hpool, \
             tc.tile_pool(name="y", bufs=3) as ypool, \
             tc.tile_pool(name="bu_ps", bufs=3, space="PSUM") as bupool, \
             tc.tile_pool(name="y_ps", bufs=5, space="PSUM") as yppool:

            warm_sb = cpool.tile([128, 512], bf16, name="warm")
            nc.gpsimd.memset(warm_sb[:], 1.0)
            BwT3 = cpool.tile([128, KT, NS], bf16, name="bw")
            BwT_sb = [BwT3[:, k, :] for k in range(KT)]
            par3 = cpool.tile([128, NT + KT], f32, name="par")
            nc.gpsimd.dma_start(out=par3[:], in_=par_d[:, :])
            lamv_sb = [par3[:, n:n + 1] for n in range(NT)]
            dvec_sb = [par3[:, NT + k:NT + k + 1] for k in range(KT)]
            CwT3 = cpool.tile([128, NT, DM], bf16, name="cw")
            CwT_sb = [CwT3[:, n, :] for n in range(NT)]

            lam_sb = [cpool.tile([128, LC], f32, name=f"lam{n}") for n in range(NT)]
            for n in range(NT):
                nc.vector.memset(lam_sb[n][:], 1.0)
                nc.vector.tensor_scalar_mul(lam_sb[n][:], lam_sb[n][:],
                                            lamv_sb[n])

            # ---- PE warmup: dummy matmuls raise the HAM clock out of the
            # low p-state while the first u chunk is still in flight
            if warm:
                warm_ps = yppool.tile([128, LC], f32, tag="y")
                for w in range(warm):
                    nc.tensor.matmul(warm_ps[:], warm_sb[:, 0:128], warm_sb[:],
                                     start=(w == 0), stop=(w == warm - 1))

            hr = [hpool.tile([128, HALF], bf16, name=f"hr_{n}") for n in range(NT)]
            hh = [hpool.tile([128, LC], bf16, name=f"hh_{n}") for n in range(NT)]

            # u tiles: index 0 is the halo chunk, 1.. are main chunks
            uC_sb = [upool.tile([128, KT, LC], bf16, name=f"uc{c}")
                     for c in range(NTOT)]

            # ---- front: BwT and halo as ONE dma_start each.  The HWDGE
            # generator emits 128 descriptors per dma_start regardless of
            # bytes, so splitting doubles the descriptor count ahead of
            # chunk1 and delays the whole dense phase.
            nc.sync.dma_start(out=BwT3[:], in_=BwT_d[:, :, :])
            nc.sync.dma_start(out=uC_sb[0][:], in_=uT_d[:, 0, :, :])
            for n in range(NT):
                bu_ps = bupool.tile([128, LC], f32, tag="bu")
                for k in range(KT):
                    nc.tensor.matmul(bu_ps[:],
                                     BwT_sb[k][:, n * 128:(n + 1) * 128],
                                     uC_sb[0][:, k, :],
                                     start=(k == 0), stop=(k == KT - 1))
                nc.vector.tensor_tensor_scan(
                    hh[n][:], lam_sb[n][:], bu_ps[:], 0.0, MULT, ADD)

            # ---- main chunks: GEMM1 -> scan -> GEMM2 -> y out.
            # GEMM2/y-mat run one chunk behind the scan chain so the next
            # scan never queues behind y work on the in-order DVE.
            def gemm2(c):
                # last chunk streams y out per-k (short tail); earlier
                # chunks coalesce all 8 k-slices into one 8KB/partition DMA
                tail = c == NLC - 1
                o = c * LC
                y_sb = ypool.tile([128, KT, LC], bf16, tag="ysb")
                for k in range(KT):
                    y_ps = yppool.tile([128, LC], f32, tag="y")
                    for n in range(NT):
                        nc.tensor.matmul(y_ps[:],
                                         CwT_sb[n][:, k * 128:(k + 1) * 128],
                                         hr[n][:, o:o + LC],
                                         start=(n == 0), stop=(n == NT - 1))
                    nc.vector.scalar_tensor_tensor(
                        y_sb[:, k, :], uC_sb[1 + c][:, k, :],
                        dvec_sb[k], y_ps[:], MULT, ADD)
                    if tail:
                        nc.scalar.dma_start(out=yT_d[:, c, k, :],
                                            in_=y_sb[:, k, :])
                if not tail:
                    nc.scalar.dma_start(out=yT_d[:, c, :, :], in_=y_sb[:])

            for c in range(NLC):
                nc.sync.dma_start(out=uC_sb[1 + c][:], in_=uT_d[:, NHC + c, :, :])
                if c == 1:
                    # GEMM2-only constant: after chunk2 in the DMA FIFO,
                    # lands before GEMM2(0) needs it
                    nc.sync.dma_start(out=CwT3[:], in_=CwT_d[:, :, :])
                o = c * LC
                for n in range(NT):
                    bu_ps = bupool.tile([128, LC], f32, tag="bu")
                    for k in range(KT):
                        nc.tensor.matmul(bu_ps[:],
                                         BwT_sb[k][:, n * 128:(n + 1) * 128],
                                         uC_sb[1 + c][:, k, :],
                                         start=(k == 0), stop=(k == KT - 1))
                    init = (hh[n][:, LC - 1:LC] if c == 0
                            else hr[n][:, o - 1:o])
                    nc.vector.tensor_tensor_scan(
                        hr[n][:, o:o + LC],
                        lam_sb[n][:], bu_ps[:], init, MULT, ADD)
                if c > 0:
                    gemm2(c - 1)
            gemm2(NLC - 1)

    nc.compile()
    return nc


def _sigmoid(x):
    return 1.0 / (1.0 + np.exp(-x))


def kernel(u, log_lambda, B_w, C_w, D):
    from concourse.bass_utils import run_bass_kernel_spmd

    if "nc" not in _CACHE:
        _CACHE["nc"] = _build()
    nc = _CACHE["nc"]

    u = np.asarray(u, dtype=np.float32)
    lam = _sigmoid(np.asarray(log_lambda, dtype=np.float64))
    # p-major layouts: [128, KT, ...] so one dma_start covers all k-tiles
    BwT = np.ascontiguousarray(
        np.asarray(B_w, np.float32).T.reshape(KT, 128, NS).transpose(1, 0, 2)
    ).astype(BF16)
    CwT = np.ascontiguousarray(
        np.asarray(C_w, np.float32).T.reshape(NT, 128, DM).transpose(1, 0, 2)
    ).astype(BF16)
    params = np.empty((128, NT + KT), dtype=np.float32)
    params[:, :NT] = lam.reshape(NT, 128).T.astype(np.float32)
    params[:, NT:] = np.asarray(D, np.float32).reshape(KT, 128).T

    in_maps = []
    for core in range(NCORES):
        b, s = core // 2, core % 2
        # [D, HALO + HALF] columns: zero halo for first-half cores
        cols = np.zeros((DM, HALO + HALF), dtype=np.float32)
        if s == 1:
            cols[:, :HALO] = u[b, HALF - HALO:HALF, :].T
        cols[:, HALO:] = u[b, s * HALF:(s + 1) * HALF, :].T
        # [p, chunk, k, l] with d = k*128 + p, col = chunk*LC + l
        uTh = np.ascontiguousarray(
            cols.reshape(KT, 128, NTOT, LC).transpose(1, 2, 0, 3)
        ).astype(BF16)
        in_maps.append({
            "uT": uTh,
            "BwT": BwT,
            "CwT": CwT,
            "params": params,
        })
    _CACHE["in_maps"] = in_maps

    def _run():
        return run_bass_kernel_spmd(nc, in_maps, core_ids=list(range(NCORES)))

    try:
        res = _run()
    except Exception:
        # a previously failed execution can wedge the backend; reset + retry
        try:
            import ctypes, jax
            jax.devices()
            lib = ctypes.CDLL("/opt/axon/libaxon_pjrt.so")
            lib.axon_reset.restype = ctypes.c_int64
            lib.axon_reset()
        except Exception:
            pass
        res = _run()

    y = np.empty((B, L, DM), dtype=np.float32)
    for core in range(NCORES):
        b, s = core // 2, core % 2
        yT = res.results[core]["yT"].astype(np.float32)   # [128, NLC, KT, LC]
        y[b, s * HALF:(s + 1) * HALF, :] = (
            yT.transpose(2, 0, 1, 3).reshape(DM, HALF).T)
    return y
